# revision 24
# baseline (speedup 1.0000x reference)
"""NeuronBasedQKV on 8 trn2 cores.

Sharding: core c -> (batch b = c//2, rank-half j = c%2).
Each core computes its batch's router/recipes fully, projects x through all
32 bases restricted to its 256-wide rank half, runs the 8 heads living in
that half, and produces a partial O-projection (contracted over its rank
half only).  Host sums the two partials per batch.
"""

import os
import numpy as np
from contextlib import ExitStack

import concourse.bass as bass
import concourse.tile as tile
from concourse import bacc, mybir
from concourse.bass_utils import run_bass_kernel_spmd
from concourse.masks import make_identity

F32 = mybir.dt.float32
BF16 = mybir.dt.bfloat16
AX = mybir.AxisListType
OP = mybir.AluOpType
AF = mybir.ActivationFunctionType
NPBF16 = mybir.dt.np(mybir.dt.bfloat16)

B, S, D = 4, 1024, 1024
NN, NB, RANK = 256, 32, 512
NH, K, DH = 16, 8, 32
HR = RANK // 2          # rank half per core
NHH = 8                 # heads per core
NT = 8                  # 128-token tiles
NDC = 8                 # 128-wide d chunks
SCALE = 1.0 / np.sqrt(DH)
NEG = -1.0e30

_cache = {}


def _classify(m):
    """m: [S,S] bool mask.  Blocks are [512 queries, 128 keys]."""
    cls = {}
    partials = []
    for qh in range(2):
        for kc in range(8):
            sub = m[qh * 512:(qh + 1) * 512, kc * 128:(kc + 1) * 128]
            if sub.all():
                cls[(qh, kc)] = ("F", -1)
            elif not sub.any():
                cls[(qh, kc)] = ("Z", -1)
            else:
                cls[(qh, kc)] = ("P", len(partials))
                partials.append(np.ascontiguousarray(sub.T).astype(NPBF16))
    return cls, partials


def _build(cls, npar):
    nc = bacc.Bacc("TRN2", target_bir_lowering=False, debug=False)

    xT32_d = nc.dram_tensor("xT32", [128, NDC, S], F32, kind="ExternalInput").ap()
    xT16_d = nc.dram_tensor("xT16", [128, NDC, S], BF16, kind="ExternalInput").ap()
    Wr_d = nc.dram_tensor("Wr", [128, NDC, NN], F32, kind="ExternalInput").ap()
    rcat_d = nc.dram_tensor("rcat", [128, 2, 128], BF16, kind="ExternalInput").ap()
    Bqk_d = nc.dram_tensor("Bqk", [NB, 128, NDC, HR], BF16, kind="ExternalInput").ap()
    Bvo_d = nc.dram_tensor("Bvo", [NB, 128, NDC, HR], BF16, kind="ExternalInput").ap()
    BvoT_d = nc.dram_tensor("BvoT", [64, 128, D], BF16, kind="ExternalInput").ap()
    mkT_d = nc.dram_tensor("mkT", [npar, 128, 512], BF16, kind="ExternalInput").ap()
    out_d = nc.dram_tensor("out", [S, D], F32, kind="ExternalOutput").ap()

    with tile.TileContext(nc) as tc, ExitStack() as ctx:
        # ---------- persistent pool ----------
        pp = ctx.enter_context(tc.tile_pool(name="persist", bufs=1))
        ident_f = pp.tile([128, 128], F32)
        make_identity(nc, ident_f)
        ident_b = pp.tile([128, 128], BF16)
        make_identity(nc, ident_b)
        ones_b = pp.tile([1, 128], BF16)
        nc.vector.memset(ones_b[:], 1.0)
        ones_f = pp.tile([1, 32], F32)
        nc.vector.memset(ones_f[:], 1.0)
        neg_t = pp.tile([128, NN], F32)
        nc.vector.memset(neg_t[:], NEG)
        tr_sb = pp.tile([128, NT, 128], F32)       # cols 0:32 Q, 32:64 K, 64:96 V, 96:128 O
        onT = pp.tile([128, 2, S], BF16)           # attn out^T: [r_local%128, r_local//128, token]
        # trO^T flattened onto partition 0; own pool so it can die before O phase
        trot_ctx = ctx.enter_context(tc.tile_pool(name="trot", bufs=1))
        trOT0 = trot_ctx.tile([1, NB, S], BF16)

        # ================= phase R: router + recipes =================
        with tc.tile_pool(name="rphase", bufs=1) as rp, \
             tc.tile_pool(name="rps", bufs=2, space=bass.MemorySpace.PSUM) as rps, \
             tc.tile_pool(name="tps", bufs=2, space=bass.MemorySpace.PSUM) as tps, \
             tc.tile_pool(name="mps", bufs=2, space=bass.MemorySpace.PSUM) as mps, \
             tc.tile_pool(name="rwork", bufs=2) as rw:
            xT32 = rp.tile([128, NDC, S], F32)
            nc.sync.dma_start(xT32[:], xT32_d[:])
            Wr = rp.tile([128, NDC, NN], F32)
            nc.sync.dma_start(Wr[:], Wr_d[:])
            rcat_sb = rp.tile([128, 2, 128], BF16)
            nc.sync.dma_start(rcat_sb[:], rcat_d[:])
            wT = rp.tile([128, 2, S], BF16)

            for t in range(NT):
                sc_ps = rps.tile([128, NN], F32)
                for dc in range(NDC):
                    nc.tensor.matmul(sc_ps[:], xT32[:, dc, t * 128:(t + 1) * 128],
                                     Wr[:, dc, :], start=(dc == 0), stop=(dc == NDC - 1))
                sc = rw.tile([128, NN], F32)
                nc.scalar.copy(sc[:], sc_ps[:])
                cur = rw.tile([128, NN], F32)
                nc.vector.tensor_copy(cur[:], sc[:])
                m1 = rw.tile([128, 1], F32)
                t8 = rw.tile([128, 1], F32)
                for i in range(K):
                    mi = m1 if i == 0 else t8
                    nc.vector.tensor_reduce(mi[:], cur[:], AX.X, OP.max)
                    if i < K - 1:
                        ge = rw.tile([128, NN], mybir.dt.uint8)
                        nc.vector.tensor_scalar(ge[:], cur[:], mi[:], None, OP.is_ge)
                        nc.vector.copy_predicated(cur[:], ge[:], neg_t[:])
                nm1 = rw.tile([128, 1], F32)
                nc.vector.tensor_scalar(nm1[:], m1[:], -1.0, None, OP.mult)
                e = rw.tile([128, NN], F32)
                nc.scalar.activation(e[:], sc[:], AF.Exp, bias=nm1[:])
                keep = rw.tile([128, NN], F32)
                nc.vector.tensor_scalar(keep[:], sc[:], t8[:], None, OP.is_ge)
                w = rw.tile([128, NN], F32)
                nc.vector.tensor_tensor(w[:], e[:], keep[:], OP.mult)
                ws = rw.tile([128, 1], F32)
                nc.vector.tensor_reduce(ws[:], w[:], AX.X, OP.add)
                wr = rw.tile([128, 1], F32)
                nc.vector.reciprocal(wr[:], ws[:])
                wb = rw.tile([128, NN], BF16)
                nc.scalar.activation(wb[:], w[:], AF.Copy, scale=wr[:])
                for kc in range(2):
                    tp = tps.tile([128, 128], BF16)
                    nc.tensor.transpose(tp[:], wb[:, kc * 128:(kc + 1) * 128], ident_b[:])
                    nc.scalar.copy(wT[:, kc, t * 128:(t + 1) * 128], tp[:])

            for t in range(NT):
                mx = mps.tile([128, 128], F32)
                for kc in range(2):
                    nc.tensor.matmul(mx[:], wT[:, kc, t * 128:(t + 1) * 128],
                                     rcat_sb[:, kc, :], start=(kc == 0), stop=(kc == 1))
                eg = rw.tile([128, 4, 32], F32)
                for g in range(4):
                    nc.scalar.activation(eg[:, g, :], mx[:, g * 32:(g + 1) * 32], AF.Exp)
                gs = rw.tile([128, 4], F32)
                nc.vector.tensor_reduce(gs[:], eg[:], AX.X, OP.add)
                gr = rw.tile([128, 4], F32)
                nc.vector.reciprocal(gr[:], gs[:])
                for g in range(4):
                    nc.scalar.activation(tr_sb[:, t, g * 32:(g + 1) * 32], eg[:, g, :],
                                         AF.Copy, scale=gr[:, g:g + 1])

        # ================= projection phases =================
        with tc.tile_pool(name="accs", bufs=1) as ap_pool:
            xT16 = ap_pool.tile([128, NDC, S], BF16)
            nc.sync.dma_start(xT16[:], xT16_d[:])
            accQ = ap_pool.tile([128, NT, HR], F32)
            accK = ap_pool.tile([128, NT, HR], F32)
            accV = ap_pool.tile([128, NT, HR], F32)
            QT = ap_pool.tile([32, NHH, S], BF16)
            KT = ap_pool.tile([32, NHH, S], BF16)
            Vx = ap_pool.tile([128, NT, NHH, 33], BF16)
            mk_sb = ap_pool.tile([128, npar, 512], BF16)
            trOT_st = ap_pool.tile([32, S], BF16)
            nc.vector.memset(Vx[:], 1.0)
            for pi in range(npar):
                nc.sync.dma_start(mk_sb[:, pi, :], mkT_d[pi])

            def proj(basis_d, accs):
                with tc.tile_pool(name="bstream", bufs=3) as bsp, \
                     tc.tile_pool(name="xps", bufs=4, space=bass.MemorySpace.PSUM) as xps:
                    for n in range(NB):
                        bt = bsp.tile([128, NDC, HR], BF16, name="bt")
                        nc.sync.dma_start(bt[:], basis_d[n])
                        for t in range(NT):
                            xp = xps.tile([128, HR], F32, name="xp")
                            for dc in range(NDC):
                                nc.tensor.matmul(xp[:], xT16[:, dc, t * 128:(t + 1) * 128],
                                                 bt[:, dc, :], start=(dc == 0),
                                                 stop=(dc == NDC - 1))
                            for acc, col0 in accs:
                                c = tr_sb[:, t, col0 + n:col0 + n + 1]
                                if n == 0:
                                    nc.vector.tensor_scalar(acc[:, t, :], xp[:], c, None,
                                                            OP.mult)
                                else:
                                    nc.vector.scalar_tensor_tensor(acc[:, t, :], xp[:], c,
                                                                   acc[:, t, :], OP.mult,
                                                                   OP.add)

            proj(Bqk_d, [(accQ, 0), (accK, 32)])
            proj(Bvo_d, [(accV, 64)])

            # ---------- attn prep: Vx fill + Q/K/trO transposes ----------
            with tc.tile_pool(name="tp2", bufs=3, space=bass.MemorySpace.PSUM) as tps2:
                for t in range(NT):
                    for h in range(NHH):
                        nc.scalar.copy(Vx[:, t, h, 0:32], accV[:, t, h * 32:(h + 1) * 32])
                for t in range(NT):
                    for h in range(NHH):
                        q_ps = tps2.tile([32, 128], F32, name="tp2")
                        nc.tensor.transpose(q_ps[:], accQ[:, t, h * 32:(h + 1) * 32],
                                            ident_f[:])
                        nc.scalar.activation(QT[:, h, t * 128:(t + 1) * 128], q_ps[:],
                                             AF.Copy, scale=float(SCALE))
                        k_ps = tps2.tile([32, 128], F32, name="tp2")
                        nc.tensor.transpose(k_ps[:], accK[:, t, h * 32:(h + 1) * 32],
                                            ident_f[:])
                        nc.scalar.copy(KT[:, h, t * 128:(t + 1) * 128], k_ps[:])
                for t in range(NT):
                    o_ps = tps2.tile([32, 128], F32, name="tp2")
                    nc.tensor.transpose(o_ps[:], tr_sb[:, t, 96:128], ident_f[:])
                    nc.scalar.copy(trOT_st[:, t * 128:(t + 1) * 128], o_ps[:])
                for n in range(NB):
                    nc.sync.dma_start(trOT0[0:1, n, :], trOT_st[n:n + 1, :])

            # ---------- attention ----------
            first_kc = {}
            last_kc = {}
            for qh in range(2):
                live = [kc for kc in range(8) if cls[(qh, kc)][0] != "Z"]
                first_kc[qh], last_kc[qh] = live[0], live[-1]
            with tc.tile_pool(name="stp", bufs=3, space=bass.MemorySpace.PSUM) as stp, \
                 tc.tile_pool(name="avp", bufs=1, space=bass.MemorySpace.PSUM) as avp, \
                 tc.tile_pool(name="bcp", bufs=1, space=bass.MemorySpace.PSUM) as bcp, \
                 tc.tile_pool(name="atw", bufs=3) as atw:
                for h in range(NHH):
                    av = avp.tile([33, S], F32, name="av")
                    for qh in range(2):
                        for kc in range(8):
                            kind, pi = cls[(qh, kc)]
                            if kind == "Z":
                                continue
                            st = stp.tile([128, 512], F32, name="st")
                            nc.tensor.matmul(st[:], KT[:, h, kc * 128:(kc + 1) * 128],
                                             QT[:, h, qh * 512:(qh + 1) * 512],
                                             start=True, stop=True)
                            pt = atw.tile([128, 512], BF16, name="pt")
                            nc.scalar.activation(pt[:], st[:], AF.Exp)
                            if kind == "P":
                                nc.vector.tensor_tensor(pt[:], pt[:], mk_sb[:, pi, :],
                                                        OP.mult)
                            nc.tensor.matmul(av[:, qh * 512:(qh + 1) * 512],
                                             Vx[:, kc, h, :], pt[:],
                                             start=(kc == first_kc[qh]),
                                             stop=(kc == last_kc[qh]))
                    recip = atw.tile([1, S], F32, name="recip")
                    nc.vector.reciprocal(recip[:], av[32:33, :])
                    bc = bcp.tile([32, S], F32, name="bc")
                    for qh in range(2):
                        nc.tensor.matmul(bc[:, qh * 512:(qh + 1) * 512], ones_f[:],
                                         recip[0:1, qh * 512:(qh + 1) * 512],
                                         start=True, stop=True)
                    bcs = atw.tile([32, S], BF16, name="bcs")
                    nc.scalar.copy(bcs[:], bc[:])
                    tmp = atw.tile([32, S], BF16, name="tmp")
                    nc.vector.tensor_tensor(tmp[:], av[0:32, :], bcs[:], OP.mult)
                    nc.sync.dma_start(onT[32 * (h % 4):32 * (h % 4) + 32, h // 4, :],
                                      tmp[:])

        # ========= broadcast phase: tbAll[p, n, t] = trO[t, n] =========
        with tc.tile_pool(name="tba", bufs=1) as tba:
            tbAll = tba.tile([128, NB, S], BF16)
            with tc.tile_pool(name="zps", bufs=2, space=bass.MemorySpace.PSUM) as zps:
                for n in range(NB):
                    b2 = zps.tile([128, S], F32, name="b2")
                    for qh in range(2):
                        nc.tensor.matmul(b2[:, qh * 512:(qh + 1) * 512], ones_b[:],
                                         trOT0[0:1, n, qh * 512:(qh + 1) * 512],
                                         start=True, stop=True)
                    nc.scalar.copy(tbAll[:, n, :], b2[:])

            # ================= O phase =================
            with tc.tile_pool(name="bvs", bufs=3) as bvs, \
                 tc.tile_pool(name="zwp", bufs=3) as zwp, \
                 tc.tile_pool(name="osb", bufs=2) as osb, \
                 tc.tile_pool(name="ops", bufs=1, space=bass.MemorySpace.PSUM) as ops:
                for g in range(2):
                    o_ps = [ops.tile([128, D], F32, name=f"op{ti}") for ti in range(4)]
                    for kc in range(64):
                        n, rc = kc // 2, kc % 2
                        bv = bvs.tile([128, D], BF16, name="bv")
                        nc.sync.dma_start(bv[:], BvoT_d[kc])
                        zrow = zwp.tile([128, S], BF16, name="zrow")
                        nc.vector.tensor_tensor(zrow[:], onT[:, rc, :],
                                                tbAll[:, n, :], OP.mult)
                        for ti in range(4):
                            t = g * 4 + ti
                            for d2 in range(2):
                                nc.tensor.matmul(
                                    o_ps[ti][:, d2 * 512:(d2 + 1) * 512],
                                    zrow[:, t * 128:(t + 1) * 128],
                                    bv[:, d2 * 512:(d2 + 1) * 512],
                                    start=(kc == 0), stop=(kc == 63))
                    for ti in range(4):
                        t = g * 4 + ti
                        o_sb = osb.tile([128, D], F32, name="osb")
                        nc.scalar.copy(o_sb[:], o_ps[ti][:])
                        nc.sync.dma_start(out_d[t * 128:(t + 1) * 128, :], o_sb[:])

    nc.compile()
    return nc


def _prep_inputs(inputs):
    x = np.asarray(inputs["x"], dtype=np.float32)
    Wro = np.asarray(inputs["W_router"], dtype=np.float32)
    rQ = np.asarray(inputs["recipe_Q"], dtype=np.float32)
    rK = np.asarray(inputs["recipe_K"], dtype=np.float32)
    rV = np.asarray(inputs["recipe_V"], dtype=np.float32)
    rO = np.asarray(inputs["recipe_O"], dtype=np.float32)
    bqk = np.asarray(inputs["basis_qk"], dtype=np.float32)
    bvo = np.asarray(inputs["basis_vo"], dtype=np.float32)

    Wr = np.ascontiguousarray(Wro.reshape(NDC, 128, NN).transpose(1, 0, 2))
    rcat = np.concatenate([rQ, rK, rV, rO], axis=1)  # [256,128]
    rcat = np.ascontiguousarray(rcat.reshape(2, 128, 128).transpose(1, 0, 2)).astype(NPBF16)

    xT32 = []
    xT16 = []
    for b in range(B):
        xt = np.ascontiguousarray(
            x[b].T.reshape(NDC, 128, S).transpose(1, 0, 2))
        xT32.append(xt)
        xT16.append(xt.astype(NPBF16))

    Bqk_j, Bvo_j, BvoT_j = [], [], []
    for j in range(2):
        rs = slice(j * HR, (j + 1) * HR)
        bq = np.ascontiguousarray(
            bqk[:, :, rs].reshape(NB, NDC, 128, HR).transpose(0, 2, 1, 3)).astype(NPBF16)
        bv = np.ascontiguousarray(
            bvo[:, :, rs].reshape(NB, NDC, 128, HR).transpose(0, 2, 1, 3)).astype(NPBF16)
        bvt = np.ascontiguousarray(
            bvo[:, :, rs].transpose(0, 2, 1).reshape(64, 128, D)).astype(NPBF16)
        Bqk_j.append(bq)
        Bvo_j.append(bv)
        BvoT_j.append(bvt)
    return xT32, xT16, Wr, rcat, Bqk_j, Bvo_j, BvoT_j


def kernel(**inputs):
    m = np.asarray(inputs["mask"]).reshape(S, S)
    cls, partials = _classify(m)
    npar = max(1, len(partials))
    mkT = (np.stack(partials) if partials
           else np.ones((1, 128, 512), dtype=NPBF16))

    key = tuple(sorted((k, v[0]) for k, v in cls.items())) + (npar,)
    if key not in _cache:
        _cache[key] = _build(cls, npar)
    nc = _cache[key]

    xT32, xT16, Wr, rcat, Bqk_j, Bvo_j, BvoT_j = _prep_inputs(inputs)

    ins = []
    for c in range(8):
        b, j = c // 2, c % 2
        ins.append({
            "xT32": xT32[b], "xT16": xT16[b], "Wr": Wr, "rcat": rcat,
            "Bqk": Bqk_j[j], "Bvo": Bvo_j[j], "BvoT": BvoT_j[j], "mkT": mkT,
        })

    kernel.last_nc = nc
    kernel.last_ins = ins
    trace = bool(int(os.environ.get("KERNEL_TRACE", "0")))
    res = run_bass_kernel_spmd(nc, ins, list(range(8)), trace=trace)
    if trace:
        kernel.last_results = res

    out = np.empty((B, S, D), dtype=np.float32)
    for b in range(B):
        out[b] = res.results[2 * b]["out"] + res.results[2 * b + 1]["out"]
    return out
